# revision 15
# baseline (speedup 1.0000x reference)
"""Distributed causal single-head attention for 8 Trainium2 NeuronCores.

Problem: x [B=4, T=4096, E=1024] f32; Wq/Wk/Wv [E, H=64] f32.
out[b] = softmax(causal(q k^T / sqrt(H))) v,  q/k/v = x[b] @ W.

Sharding: core = (batch b = core//2, parity o = core%2). Each core computes
the output rows of the interleaved 512-row chunks {2J+o : J=0..3} of batch b.
The host ships x[b]^T (bf16, tile-blocked for contiguous DMA) with columns
permuted to [own-chunks | partner-chunks] so all 8 cores run one identical
(SPMD) instruction stream; parity enters only through data (a scalar mask
input). On this stack DRAM streaming bandwidth dominates everything, so all
wire tensors are bf16 and causal masks are generated on-device.

Within a core:
  - q/k/v projections as bf16 matmuls (qk-pack [Wq|Wk], kv-pack [Wk|Wv]),
  - scoresT[k,q] via row-packed K=64 matmul pairs,
  - exp on ScalarE (PSUM f32 -> SBUF bf16), causal masks multiplied in,
  - PV as bf16 matmuls with an extra ones-row in v^T (rides through the
    xbar transpose) producing softmax denominators for free,
  - normalization via f32r reciprocal + K=1 broadcast matmul.
"""

import sys

sys.path.insert(0, "/opt/trn_rl_repo")

import numpy as np

import concourse.bass as bass  # noqa: F401
import concourse.tile as tile
from concourse import bacc, mybir
from concourse.bass_utils import run_bass_kernel_spmd

F32 = mybir.dt.float32
F32R = mybir.dt.float32r
BF16 = mybir.dt.bfloat16

B, T, E, H = 4, 4096, 1024, 64
N_CORES = 8
SCALE = float(H) ** -0.5


def build_attention(Eb=E, Tg=T, CH=512, n_loop=1, n_cores=N_CORES):
    """Per-core inputs:
    xb [2*NCH, 128, NE*CH] bf16  (xb[b,p,e*CH+c] = x_local[b*CH+c, e*128+p]),
    w  [NE, 128, 192] bf16       ([Wq | Wk | Wv] row-tiled),
    pm [128, 1] f32, ones [1, H] f32r  ->  outT [H, Tl] bf16.
    """
    Tl = Tg // 2
    TPC = CH // 128          # k-tiles per chunk
    NCH = Tl // CH           # local q-chunks per core
    NTl = Tl // 128          # own k-tiles
    NTg = 2 * NTl            # all k-tiles (own + partner)
    NE = Eb // 128           # contraction tiles
    NB = 2 * NCH             # column blocks (own + partner)
    VW = 128                 # padded v row width (xbar-safe)

    nc = bacc.Bacc("TRN2", target_bir_lowering=False, debug=False,
                   num_devices=n_cores)
    xb_ext = nc.dram_tensor("xb", [NB, 128, NE * CH], BF16, kind="ExternalInput")
    w_ext = nc.dram_tensor("w", [NE, 128, 192], BF16, kind="ExternalInput")
    pm_ext = nc.dram_tensor("pm", [128, 1], F32, kind="ExternalInput")
    ones_ext = nc.dram_tensor("ones", [1, H], F32R, kind="ExternalInput")
    outT_ext = nc.dram_tensor("outT", [H, Tl], BF16, kind="ExternalOutput")

    with tile.TileContext(nc) as tc:
        with (
            tc.tile_pool(name="const", bufs=1) as cpool,
            tc.tile_pool(name="big", bufs=1) as bigpool,
            tc.tile_pool(name="xs", bufs=4) as xpool,
            tc.tile_pool(name="work", bufs=4) as wpool,
            tc.tile_pool(name="psA", bufs=2, space="PSUM") as psA,
            tc.tile_pool(name="psB", bufs=2, space="PSUM") as psB,
            tc.tile_pool(name="psP", bufs=2, space="PSUM") as psP,
            tc.tile_pool(name="psO", bufs=2, space="PSUM") as psO,
        ):
            # ---- constants ----
            w = cpool.tile([128, NE, 192], BF16, tag="w")
            tri = cpool.tile([128, TPC, CH], BF16, tag="tri")
            pm = cpool.tile([128, 1], F32, tag="pm")
            ones64 = cpool.tile([1, H], F32R, tag="ones64")
            for e in range(NE):
                nc.sync.dma_start(out=w[:, e, :], in_=w_ext.ap()[e])
            nc.sync.dma_start(out=pm[:, :], in_=pm_ext.ap())
            nc.sync.dma_start(out=ones64[:, :], in_=ones_ext.ap())
            # tri[i][p, f] = 1.0 if 128*i + p <= f else 0.0
            for i in range(TPC):
                nc.gpsimd.memset(tri[:, i, :], 0.0)
                nc.gpsimd.affine_select(
                    out=tri[:, i, :], in_=tri[:, i, :],
                    compare_op=mybir.AluOpType.is_gt, fill=1.0,
                    base=128 * i, pattern=[[-1, CH]], channel_multiplier=1)

            def body(_iv=0, unroll=1):
                qdup = bigpool.tile([128, Tl], BF16, tag="qdup")
                kdup = bigpool.tile([128, Tg], BF16, tag="kdup")
                vT_sb = bigpool.tile([80, Tg], BF16, tag="vT_sb")
                nc.vector.memset(vT_sb[64:80, :], 1.0)
                v_nat = bigpool.tile([128, NTg, VW], BF16, tag="v_nat")

                # ---- projections ----
                def load_block(b):
                    xt = xpool.tile([128, NE, CH], BF16, tag="xT")
                    nc.sync.dma_start(
                        out=xt[:, :, :],
                        in_=xb_ext.ap()[b].rearrange("p (e c) -> p e c", e=NE))
                    return xt

                for b in range(NB):
                    own = b < NCH
                    xt = load_block(b)
                    cols = slice(b * CH, (b + 1) * CH)
                    ps = psP.tile([128, CH], F32, tag="proj")
                    woff = 0 if own else 64  # [Wq|Wk] or [Wk|Wv]
                    for e in range(NE):
                        nc.tensor.matmul(ps[:], w[:, e, woff:woff + 128],
                                         xt[:, e, :],
                                         start=(e == 0), stop=(e == NE - 1))
                    if own:
                        nc.vector.tensor_copy(qdup[0:64, cols], ps[0:64, :])
                        nc.vector.tensor_copy(qdup[64:128, cols], ps[0:64, :])
                        nc.vector.tensor_copy(kdup[0:64, cols], ps[64:128, :])
                        nc.vector.tensor_copy(kdup[64:128, cols], ps[64:128, :])
                        vps = psP.tile([128, CH], F32, tag="proj")
                        for e in range(NE):
                            nc.tensor.matmul(vps[0:64, :], w[:, e, 128:192],
                                             xt[:, e, :],
                                             start=(e == 0), stop=(e == NE - 1))
                        nc.vector.tensor_copy(vT_sb[0:64, cols], vps[0:64, :])
                    else:
                        nc.vector.tensor_copy(kdup[0:64, cols], ps[0:64, :])
                        nc.vector.tensor_copy(kdup[64:128, cols], ps[0:64, :])
                        nc.vector.tensor_copy(vT_sb[0:64, cols], ps[64:128, :])

                # v natural (+ones row): per-tile xbar transpose [80,128]->[128,80]
                # NB: issued on the scalar HWDGE engine - concurrent DMACopy and
                # DMATranspose on one engine corrupts the xbar output.
                for t in range(NTg):
                    nc.scalar.dma_start(out=v_nat[:, t, 0:80],
                                        in_=vT_sb[0:80, t * 128:(t + 1) * 128],
                                        transpose=True)

                # ---- attention ----
                for J in range(NCH):
                    qb = slice(J * CH, (J + 1) * CH)
                    entries = []
                    for c in range(J):
                        for i in range(TPC):
                            entries.append((TPC * c + i, None))
                    for c in range(J):
                        for i in range(TPC):
                            entries.append((NTl + TPC * c + i, None))
                    for i in range(TPC):
                        entries.append((TPC * J + i, ("tri", i)))
                    for i in range(TPC):
                        entries.append((NTl + TPC * J + i, ("pm", 0)))

                    n = len(entries)
                    o_ps = psO.tile([H + 1, CH], F32, tag="o")

                    def do_half(idx, ps):
                        t, mask = entries[idx]
                        ex = wpool.tile([128, CH], BF16, tag="ex")
                        nc.scalar.activation(ex[:], ps[:],
                                             mybir.ActivationFunctionType.Exp,
                                             scale=SCALE)
                        if mask is not None:
                            kind, i = mask
                            if kind == "tri":
                                nc.vector.tensor_mul(ex[:], ex[:], tri[:, i, :])
                            else:
                                nc.vector.tensor_scalar_mul(ex[:], ex[:],
                                                            pm[:, :])
                        nc.tensor.matmul(o_ps[:], v_nat[:, t, 0:H + 1], ex[:],
                                         start=(idx == 0), stop=(idx == n - 1))

                    for p in range(n // 2):
                        t0 = entries[2 * p][0]
                        t1 = entries[2 * p + 1][0]
                        ps_a = psA.tile([128, CH], F32, tag="sa")
                        ps_b = psB.tile([128, CH], F32, tag="sb")
                        c0 = slice(t0 * 128, t0 * 128 + 128)
                        c1 = slice(t1 * 128, t1 * 128 + 128)
                        nc.tensor.matmul(ps_a[:], kdup[0:64, c0],
                                         qdup[0:64, qb], start=True, stop=True)
                        nc.tensor.matmul(ps_b[:], kdup[64:128, c1],
                                         qdup[64:128, qb], start=True, stop=True)
                        do_half(2 * p, ps_a)
                        do_half(2 * p + 1, ps_b)

                    # normalize and store
                    recip = wpool.tile([1, CH], F32R, tag="recip")
                    with nc.allow_low_precision(reason="f32r recip for norm"):
                        nc.vector.reciprocal(recip[:], o_ps[H:H + 1, :])
                    rb_ps = psA.tile([H, CH], F32, tag="sa")
                    nc.tensor.matmul(rb_ps[:], ones64[:], recip[:],
                                     start=True, stop=True)
                    o_sb0 = wpool.tile([H, CH], F32, tag="osb0")
                    nc.vector.tensor_copy(o_sb0[:], o_ps[0:H, :])
                    o_sb = wpool.tile([H, CH], BF16, tag="osb")
                    nc.vector.tensor_mul(o_sb[:], o_sb0[:], rb_ps[:])
                    nc.sync.dma_start(out=outT_ext.ap()[:, qb], in_=o_sb[:])

            if n_loop == 1:
                body()
            else:
                with tc.For_i(0, n_loop, 1) as iv:
                    body(iv)

    nc.compile()
    return nc


# ---------------- host-side shard / unshard ----------------

def make_in_maps(x, Wq, Wk, Wv, Tg=T, CH=512):
    import ml_dtypes
    Tl = Tg // 2
    NCH = Tl // CH
    NB = 2 * NCH
    NE = np.asarray(Wq).shape[0] // 128
    x = np.asarray(x)
    w_all = np.concatenate([np.asarray(Wq), np.asarray(Wk), np.asarray(Wv)],
                           axis=1).astype(ml_dtypes.bfloat16)     # [E, 192]
    w_tiled = np.ascontiguousarray(w_all.reshape(NE, 128, 192))

    in_maps = []
    for core in range(N_CORES):
        b, o = core // 2, core % 2
        own = [2 * J + o for J in range(NCH)]
        par = [2 * J + (1 - o) for J in range(NCH)]
        xl = np.concatenate([x[b, g * CH:(g + 1) * CH, :] for g in own + par],
                            axis=0)                               # [Tg, E]
        # xb[blk, p, e*CH + c] = xl[blk*CH + c, e*128 + p]
        xb = xl.reshape(NB, CH, NE, 128).transpose(0, 3, 2, 1)
        xb = np.ascontiguousarray(
            xb.reshape(NB, 128, NE * CH).astype(ml_dtypes.bfloat16))
        pmv = np.full((128, 1), 1.0 if o == 1 else 0.0, np.float32)
        in_maps.append({"xb": xb, "w": w_tiled, "pm": pmv,
                        "ones": np.ones((1, H), np.float32)})
    return in_maps


def unshard_out(results, Tg=T, CH=512):
    Tl = Tg // 2
    NCH = Tl // CH
    out = np.zeros((B, Tg, H), np.float32)
    for core in range(N_CORES):
        b, o = core // 2, core % 2
        outT = np.asarray(results[core]["outT"]).astype(np.float32)
        for J in range(NCH):
            g = 2 * J + o
            out[b, g * CH:(g + 1) * CH, :] = outT[:, J * CH:(J + 1) * CH].T
    return out


_cached_nc = None


def kernel(x, Wq, Wk, Wv):
    global _cached_nc
    if _cached_nc is None:
        _cached_nc = build_attention()
    in_maps = make_in_maps(x, Wq, Wk, Wv)
    res = run_bass_kernel_spmd(_cached_nc, in_maps, core_ids=list(range(N_CORES)))
    return unshard_out(res.results)


# revision 16
# speedup vs baseline: 1.1013x; 1.1013x over previous
"""Distributed causal single-head attention for 8 Trainium2 NeuronCores.

Problem: x [B=4, T=4096, E=1024] f32; Wq/Wk/Wv [E, H=64] f32.
out[b] = softmax(causal(q k^T / sqrt(H))) v,  q/k/v = x[b] @ W.

Sharding: core = (batch b = core//2, parity o = core%2). Each core computes
the output rows of the interleaved 512-row chunks {2J+o : J=0..3} of batch b.
The host ships x[b]^T (bf16, tile-blocked for contiguous DMA) with columns
permuted to [own-chunks | partner-chunks] so all 8 cores run one identical
(SPMD) instruction stream; parity enters only through data (a scalar mask
input). On this stack DRAM streaming bandwidth dominates everything, so all
wire tensors are bf16 and causal masks are generated on-device.

Within a core:
  - q/k/v projections as bf16 matmuls (qk-pack [Wq|Wk], kv-pack [Wk|Wv]),
  - scoresT[k,q] via row-packed K=64 matmul pairs,
  - exp on ScalarE (PSUM f32 -> SBUF bf16), causal masks multiplied in,
  - PV as bf16 matmuls with an extra ones-row in v^T (rides through the
    xbar transpose) producing softmax denominators for free,
  - normalization via f32r reciprocal + K=1 broadcast matmul.
"""

import sys

sys.path.insert(0, "/opt/trn_rl_repo")

import numpy as np

import concourse.bass as bass  # noqa: F401
import concourse.tile as tile
from concourse import bacc, mybir
from concourse.bass_utils import run_bass_kernel_spmd

F32 = mybir.dt.float32
F32R = mybir.dt.float32r
BF16 = mybir.dt.bfloat16

B, T, E, H = 4, 4096, 1024, 64
N_CORES = 8
SCALE = float(H) ** -0.5


def build_attention(Eb=E, Tg=T, CH=512, n_loop=1, n_cores=N_CORES):
    """Per-core inputs:
    xb [2*NCH, 128, NE*CH] bf16  (xb[b,p,e*CH+c] = x_local[b*CH+c, e*128+p]),
    w  [NE, 128, 192] bf16       ([Wq | Wk | Wv] row-tiled),
    pm [128, 1] f32, ones [1, H] f32r  ->  outT [H, Tl] bf16.
    """
    Tl = Tg // 2
    TPC = CH // 128          # k-tiles per chunk
    NCH = Tl // CH           # local q-chunks per core
    NTl = Tl // 128          # own k-tiles
    NTg = 2 * NTl            # all k-tiles (own + partner)
    NE = Eb // 128           # contraction tiles
    NB = 2 * NCH             # column blocks (own + partner)
    VW = 128                 # padded v row width (xbar-safe)

    nc = bacc.Bacc("TRN2", target_bir_lowering=False, debug=False,
                   num_devices=n_cores)
    xb_ext = nc.dram_tensor("xb", [NB, 128, NE * CH], BF16, kind="ExternalInput")
    w_ext = nc.dram_tensor("w", [NE, 128, 192], BF16, kind="ExternalInput")
    pm_ext = nc.dram_tensor("pm", [128, 1], F32, kind="ExternalInput")
    ones_ext = nc.dram_tensor("ones", [1, H], F32R, kind="ExternalInput")
    outT_ext = nc.dram_tensor("outT", [H, Tl], BF16, kind="ExternalOutput")

    with tile.TileContext(nc) as tc:
        with (
            tc.tile_pool(name="const", bufs=1) as cpool,
            tc.tile_pool(name="big", bufs=1) as bigpool,
            tc.tile_pool(name="xs", bufs=4) as xpool,
            tc.tile_pool(name="work", bufs=4) as wpool,
            tc.tile_pool(name="psA", bufs=2, space="PSUM") as psA,
            tc.tile_pool(name="psB", bufs=2, space="PSUM") as psB,
            tc.tile_pool(name="psP", bufs=2, space="PSUM") as psP,
            tc.tile_pool(name="psO", bufs=2, space="PSUM") as psO,
        ):
            # ---- constants ----
            w = cpool.tile([128, NE, 192], BF16, tag="w")
            tri = cpool.tile([128, TPC, CH], BF16, tag="tri")
            pm = cpool.tile([128, 1], F32, tag="pm")
            ones64 = cpool.tile([1, H], F32R, tag="ones64")
            for e in range(NE):
                nc.sync.dma_start(out=w[:, e, :], in_=w_ext.ap()[e])
            nc.sync.dma_start(out=pm[:, :], in_=pm_ext.ap())
            nc.sync.dma_start(out=ones64[:, :], in_=ones_ext.ap())
            # tri[i][p, f] = 1.0 if 128*i + p <= f else 0.0
            for i in range(TPC):
                nc.gpsimd.memset(tri[:, i, :], 0.0)
                nc.gpsimd.affine_select(
                    out=tri[:, i, :], in_=tri[:, i, :],
                    compare_op=mybir.AluOpType.is_gt, fill=1.0,
                    base=128 * i, pattern=[[-1, CH]], channel_multiplier=1)

            def body(_iv=0, unroll=1):
                qdup = bigpool.tile([128, Tl], BF16, tag="qdup")
                kdup = bigpool.tile([128, Tg], BF16, tag="kdup")
                vT_sb = bigpool.tile([80, Tg], BF16, tag="vT_sb")
                nc.vector.memset(vT_sb[64:80, :], 1.0)
                v_nat = bigpool.tile([128, NTg, VW], BF16, tag="v_nat")

                # ---- projections ----
                def load_block(b):
                    xt = xpool.tile([128, NE, CH], BF16, tag="xT")
                    # split the big x loads across the HWDGE (sync) and SWDGE
                    # (gpsimd) DMA paths so they can proceed in parallel
                    eng = nc.sync if b % 2 == 0 else nc.gpsimd
                    eng.dma_start(
                        out=xt[:, :, :],
                        in_=xb_ext.ap()[b].rearrange("p (e c) -> p e c", e=NE))
                    return xt

                for b in range(NB):
                    own = b < NCH
                    xt = load_block(b)
                    cols = slice(b * CH, (b + 1) * CH)
                    ps = psP.tile([128, CH], F32, tag="proj")
                    woff = 0 if own else 64  # [Wq|Wk] or [Wk|Wv]
                    for e in range(NE):
                        nc.tensor.matmul(ps[:], w[:, e, woff:woff + 128],
                                         xt[:, e, :],
                                         start=(e == 0), stop=(e == NE - 1))
                    if own:
                        nc.vector.tensor_copy(qdup[0:64, cols], ps[0:64, :])
                        nc.vector.tensor_copy(qdup[64:128, cols], ps[0:64, :])
                        nc.vector.tensor_copy(kdup[0:64, cols], ps[64:128, :])
                        nc.vector.tensor_copy(kdup[64:128, cols], ps[64:128, :])
                        vps = psP.tile([128, CH], F32, tag="proj")
                        for e in range(NE):
                            nc.tensor.matmul(vps[0:64, :], w[:, e, 128:192],
                                             xt[:, e, :],
                                             start=(e == 0), stop=(e == NE - 1))
                        nc.vector.tensor_copy(vT_sb[0:64, cols], vps[0:64, :])
                    else:
                        nc.vector.tensor_copy(kdup[0:64, cols], ps[0:64, :])
                        nc.vector.tensor_copy(kdup[64:128, cols], ps[0:64, :])
                        nc.vector.tensor_copy(vT_sb[0:64, cols], ps[64:128, :])

                # v natural (+ones row): per-tile xbar transpose [80,128]->[128,80]
                # NB: issued on the scalar HWDGE engine - concurrent DMACopy and
                # DMATranspose on one engine corrupts the xbar output.
                for t in range(NTg):
                    nc.scalar.dma_start(out=v_nat[:, t, 0:80],
                                        in_=vT_sb[0:80, t * 128:(t + 1) * 128],
                                        transpose=True)

                # ---- attention ----
                for J in range(NCH):
                    qb = slice(J * CH, (J + 1) * CH)
                    entries = []
                    for c in range(J):
                        for i in range(TPC):
                            entries.append((TPC * c + i, None))
                    for c in range(J):
                        for i in range(TPC):
                            entries.append((NTl + TPC * c + i, None))
                    for i in range(TPC):
                        entries.append((TPC * J + i, ("tri", i)))
                    for i in range(TPC):
                        entries.append((NTl + TPC * J + i, ("pm", 0)))

                    n = len(entries)
                    o_ps = psO.tile([H + 1, CH], F32, tag="o")

                    def do_half(idx, ps):
                        t, mask = entries[idx]
                        ex = wpool.tile([128, CH], BF16, tag="ex")
                        nc.scalar.activation(ex[:], ps[:],
                                             mybir.ActivationFunctionType.Exp,
                                             scale=SCALE)
                        if mask is not None:
                            kind, i = mask
                            if kind == "tri":
                                nc.vector.tensor_mul(ex[:], ex[:], tri[:, i, :])
                            else:
                                nc.vector.tensor_scalar_mul(ex[:], ex[:],
                                                            pm[:, :])
                        nc.tensor.matmul(o_ps[:], v_nat[:, t, 0:H + 1], ex[:],
                                         start=(idx == 0), stop=(idx == n - 1))

                    for p in range(n // 2):
                        t0 = entries[2 * p][0]
                        t1 = entries[2 * p + 1][0]
                        ps_a = psA.tile([128, CH], F32, tag="sa")
                        ps_b = psB.tile([128, CH], F32, tag="sb")
                        c0 = slice(t0 * 128, t0 * 128 + 128)
                        c1 = slice(t1 * 128, t1 * 128 + 128)
                        nc.tensor.matmul(ps_a[:], kdup[0:64, c0],
                                         qdup[0:64, qb], start=True, stop=True)
                        nc.tensor.matmul(ps_b[:], kdup[64:128, c1],
                                         qdup[64:128, qb], start=True, stop=True)
                        do_half(2 * p, ps_a)
                        do_half(2 * p + 1, ps_b)

                    # normalize and store
                    recip = wpool.tile([1, CH], F32R, tag="recip")
                    with nc.allow_low_precision(reason="f32r recip for norm"):
                        nc.vector.reciprocal(recip[:], o_ps[H:H + 1, :])
                    rb_ps = psA.tile([H, CH], F32, tag="sa")
                    nc.tensor.matmul(rb_ps[:], ones64[:], recip[:],
                                     start=True, stop=True)
                    o_sb0 = wpool.tile([H, CH], F32, tag="osb0")
                    nc.vector.tensor_copy(o_sb0[:], o_ps[0:H, :])
                    o_sb = wpool.tile([H, CH], BF16, tag="osb")
                    nc.vector.tensor_mul(o_sb[:], o_sb0[:], rb_ps[:])
                    nc.sync.dma_start(out=outT_ext.ap()[:, qb], in_=o_sb[:])

            if n_loop == 1:
                body()
            else:
                with tc.For_i(0, n_loop, 1) as iv:
                    body(iv)

    nc.compile()
    return nc


# ---------------- host-side shard / unshard ----------------

def make_in_maps(x, Wq, Wk, Wv, Tg=T, CH=512):
    import ml_dtypes
    Tl = Tg // 2
    NCH = Tl // CH
    NB = 2 * NCH
    NE = np.asarray(Wq).shape[0] // 128
    x = np.asarray(x)
    w_all = np.concatenate([np.asarray(Wq), np.asarray(Wk), np.asarray(Wv)],
                           axis=1).astype(ml_dtypes.bfloat16)     # [E, 192]
    w_tiled = np.ascontiguousarray(w_all.reshape(NE, 128, 192))

    in_maps = []
    for core in range(N_CORES):
        b, o = core // 2, core % 2
        own = [2 * J + o for J in range(NCH)]
        par = [2 * J + (1 - o) for J in range(NCH)]
        xl = np.concatenate([x[b, g * CH:(g + 1) * CH, :] for g in own + par],
                            axis=0)                               # [Tg, E]
        # xb[blk, p, e*CH + c] = xl[blk*CH + c, e*128 + p]
        xb = xl.reshape(NB, CH, NE, 128).transpose(0, 3, 2, 1)
        xb = np.ascontiguousarray(
            xb.reshape(NB, 128, NE * CH).astype(ml_dtypes.bfloat16))
        pmv = np.full((128, 1), 1.0 if o == 1 else 0.0, np.float32)
        in_maps.append({"xb": xb, "w": w_tiled, "pm": pmv,
                        "ones": np.ones((1, H), np.float32)})
    return in_maps


def unshard_out(results, Tg=T, CH=512):
    Tl = Tg // 2
    NCH = Tl // CH
    out = np.zeros((B, Tg, H), np.float32)
    for core in range(N_CORES):
        b, o = core // 2, core % 2
        outT = np.asarray(results[core]["outT"]).astype(np.float32)
        for J in range(NCH):
            g = 2 * J + o
            out[b, g * CH:(g + 1) * CH, :] = outT[:, J * CH:(J + 1) * CH].T
    return out


_cached_nc = None


def kernel(x, Wq, Wk, Wv):
    global _cached_nc
    if _cached_nc is None:
        _cached_nc = build_attention()
    in_maps = make_in_maps(x, Wq, Wk, Wv)
    res = run_bass_kernel_spmd(_cached_nc, in_maps, core_ids=list(range(N_CORES)))
    return unshard_out(res.results)
